# revision 13
# baseline (speedup 1.0000x reference)
"""MoE ConditionalFeedForward (SwiGLU, top-2 of 8 experts) on 8 TRN2 NeuronCores.

Strategy: expert-parallel. Core e owns expert e's weights (w1/w2/w3 slices).
The host routes tokens: for each expert, gather the tokens assigned to it
(padded to CAP), each core computes
    y = (silu(x @ w1[e].T) * (x @ w3[e].T)) @ w2[e].T
densely for its gathered tokens, and the host scatters rows back into the
[T, A, D] output.

Per-core kernel layout (all host-pretransposed so every DMA is contiguous):
  xt  [128, 8*CAP]    xt[p, k*CAP+j] = x_g[j, k*128+p]     (tokens, transposed)
  w13 [22, 128, 2048] w13[it,p,k*128+c] = w1[e][it*128+c, k*128+p], w3 at +1024
  w2s [22, 128, 1024] w2s[it,p,d] = w2[e][d, it*128+p]
  yt  [128, 8*CAP]    f32, yt[p, k*CAP+j] = y_g[j, k*128+p] (output, transposed)

Phase A (per i-tile it of 22): h1T/h3T [128(i), CAP] = sum_k wT @ x tiles in
PSUM, then hT = silu(h1)*h3 into SBUF. Phase B (transposed): yT[d-tile] [128,
CAP] accumulated over the 22 i-tiles in PSUM (8 banks, one per d-tile), with
the w2 128x128 tile stationary and hT moving.
"""

import numpy as np
from contextlib import ExitStack

import concourse.bass as bass
import concourse.bacc as bacc
import concourse.mybir as mybir
import concourse.tile as tile
from concourse.bass_utils import run_bass_kernel_spmd

E, I, D = 8, 2816, 1024
N_CORES = 8
NI, ND = I // 128, D // 128  # 22, 8

# storage dtype for weights/activations on-device: "bfloat16" (half HBM
# traffic, full PE rate) or "float32" (matmuls run as float32r, 2 cyc/row)
DT_NAME = "bfloat16"

_PROG_CACHE: dict = {}


def _build_program(cap: int, dt_name: str):
    DT = mybir.dt.float32r if dt_name == "float32" else getattr(mybir.dt, dt_name)
    f32 = mybir.dt.float32
    NP = NI // 2  # w13/w2 DMAs batched as i-tile pairs for >=1MB transfers
    nc = bacc.Bacc("TRN2", target_bir_lowering=False, debug=False)
    xt = nc.dram_tensor("xt", [128, ND * cap], DT, kind="ExternalInput").ap()
    w13 = nc.dram_tensor("w13", [NP, 128, 4 * D], DT, kind="ExternalInput").ap()
    w2s = nc.dram_tensor("w2s", [NP, 128, 2 * D], DT, kind="ExternalInput").ap()
    yt = nc.dram_tensor("yt", [128, ND * cap], f32, kind="ExternalOutput").ap()
    warm_out = nc.dram_tensor("warm_out", [128, 16], f32, kind="ExternalOutput").ap()

    with tile.TileContext(nc) as tc, ExitStack() as ctx:
        warmp = ctx.enter_context(tc.tile_pool(name="warm", bufs=1))
        xp = ctx.enter_context(tc.tile_pool(name="x", bufs=1))
        w13p = ctx.enter_context(tc.tile_pool(name="w13", bufs=3))
        hp = ctx.enter_context(tc.tile_pool(name="h", bufs=NI))
        silp = ctx.enter_context(tc.tile_pool(name="sil", bufs=2))
        w2p = ctx.enter_context(tc.tile_pool(name="w2", bufs=1))
        yp = ctx.enter_context(tc.tile_pool(name="y", bufs=1))

        # x first on the Sync ring (0.6MB, lands before w13[0]); w13 stream
        # (critical path) follows on the same ring, split into per-i-tile
        # halves so the first matmuls only wait for 512KB. Emitted first for
        # top scheduler priority.
        xsb = xp.tile([128, ND * cap], DT)
        nc.sync.dma_start(xsb[:], xt[:])
        w13ts = []
        for j in range(3):
            wt = w13p.tile([128, 4 * D], DT, tag="w13", name=f"w13_{j}")
            nc.sync.dma_start(wt[:, 0 : 2 * D], w13[j][:, 0 : 2 * D])
            nc.sync.dma_start(wt[:, 2 * D :], w13[j][:, 2 * D :])
            w13ts.append(wt)

        # PE warmup: 8 matmuls on a zeroed tile, no DMA dependency, so the
        # HAM clock-gate is released during the initial weight-DMA window and
        # the real matmuls start at 2.4GHz.
        with tc.tile_pool(name="warmps", bufs=1, space="PSUM") as warmps:
            wtile = warmp.tile([128, 640], DT)
            nc.gpsimd.memset(wtile[:], 0.0)
            wps = warmps.tile([128, 512], f32)
            n_warm = 8
            for i in range(n_warm):
                nc.tensor.matmul(
                    wps[:],
                    wtile[:, 0:128],
                    wtile[:, 128:640],
                    start=(i == 0),
                    stop=(i == n_warm - 1),
                )
            wsc = warmp.tile([128, 16], f32)
            nc.vector.tensor_copy(wsc[:], wps[:, 0:16])
            nc.gpsimd.dma_start(warm_out[:], wsc[:])

        # w2 pair tiles on the Scalar ring: pairs 0-5 paced through phase A
        # (needed when phase B starts), pairs 6-10 paced through phase B's
        # first half, keeping phase A's DMA window under the HBM ceiling
        w2ts = [
            w2p.tile([128, 2 * D], DT, tag=f"w2_{j}", name=f"w2_{j}")
            for j in range(NP)
        ]

        hts = []
        with tc.tile_pool(name="hps", bufs=2, space="PSUM") as hps:
            for j in range(NP):
                if j < 3:
                    wt = w13ts[j]
                else:
                    wt = w13p.tile([128, 4 * D], DT, tag="w13", name=f"w13_{j}")
                    nc.sync.dma_start(wt[:, 0 : 2 * D], w13[j][:, 0 : 2 * D])
                    nc.sync.dma_start(wt[:, 2 * D :], w13[j][:, 2 * D :])
                if j % 2 == 0 and j // 2 < 6:
                    nc.scalar.dma_start(w2ts[j // 2][:], w2s[j // 2])
                for half in range(2):
                    base = half * 2 * D
                    h1 = hps.tile([128, cap], f32, tag="h1", name="h1")
                    h3 = hps.tile([128, cap], f32, tag="h3", name="h3")
                    for k in range(ND):
                        nc.tensor.matmul(
                            h1[:],
                            wt[:, base + k * 128 : base + (k + 1) * 128],
                            xsb[:, k * cap : (k + 1) * cap],
                            start=(k == 0),
                            stop=(k == ND - 1),
                        )
                    for k in range(ND):
                        nc.tensor.matmul(
                            h3[:],
                            wt[:, base + D + k * 128 : base + D + (k + 1) * 128],
                            xsb[:, k * cap : (k + 1) * cap],
                            start=(k == 0),
                            stop=(k == ND - 1),
                        )
                    sil = silp.tile([128, cap], f32)
                    nc.scalar.activation(
                        sil[:], h1[:], mybir.ActivationFunctionType.Silu
                    )
                    ht = hp.tile([128, cap], DT)
                    nc.vector.tensor_mul(ht[:], sil[:], h3[:])
                    hts.append(ht)

        # Phase B: yT[d-tile][128, cap] += w2tile.T @ hT (w2 stationary),
        # k-major within each it-half so each d-tile's PSUM drain overlaps the
        # remaining matmuls; two it-halves so w2 pairs 6-10 stream during the
        # first half
        NH = NI // 2  # 11
        with tc.tile_pool(name="yps", bufs=1, space="PSUM") as yps:
            ypt = [
                yps.tile([128, cap], f32, tag=f"yps_{k}", name=f"yps_{k}")
                for k in range(ND)
            ]
            ysb = yp.tile([128, ND * cap], f32)
            for k in range(ND):
                if 6 + k <= 10:
                    nc.scalar.dma_start(w2ts[6 + k][:], w2s[6 + k])
                for it in range(NH):
                    nc.tensor.matmul(
                        ypt[k][:],
                        w2ts[it // 2][:, (it % 2) * D + k * 128 : (it % 2) * D + (k + 1) * 128],
                        hts[it][:],
                        start=(it == 0),
                        stop=False,
                    )
            for k in range(ND):
                for it in range(NH, NI):
                    nc.tensor.matmul(
                        ypt[k][:],
                        w2ts[it // 2][:, (it % 2) * D + k * 128 : (it % 2) * D + (k + 1) * 128],
                        hts[it][:],
                        start=False,
                        stop=(it == NI - 1),
                    )
                dst = ysb[:, k * cap : (k + 1) * cap]
                nc.vector.tensor_copy(dst, ypt[k][:])
                nc.sync.dma_start(yt[:, k * cap : (k + 1) * cap], dst)

    nc.compile()
    return nc


def _get_program(cap: int, dt_name: str):
    key = (cap, dt_name)
    if key not in _PROG_CACHE:
        _PROG_CACHE[key] = _build_program(cap, dt_name)
    return _PROG_CACHE[key]


def _np_dt(dt_name: str):
    if dt_name == "float32":
        return np.float32
    import ml_dtypes

    return ml_dtypes.bfloat16


def _prep_weights(w1, w3, w2, dt_name):
    """Per-expert pretransposed/tiled weight arrays (see module docstring)."""
    npdt = _np_dt(dt_name)
    w13_all, w2s_all = [], []
    for e in range(E):
        # [I, D] -> [it, c, k, p] -> [it, p, k, c] -> [it, 128, 1024]
        a1 = w1[e].reshape(NI, 128, ND, 128).transpose(0, 3, 2, 1).reshape(NI, 128, D)
        a3 = w3[e].reshape(NI, 128, ND, 128).transpose(0, 3, 2, 1).reshape(NI, 128, D)
        # pairs of i-tiles: [11, 128, 4096] = [w1|w3] for it=2j then it=2j+1
        a13 = np.concatenate([a1, a3], axis=2).reshape(NI // 2, 2, 128, 2 * D)
        w13_all.append(
            np.ascontiguousarray(a13.transpose(0, 2, 1, 3)).reshape(
                NI // 2, 128, 4 * D
            ).astype(npdt)
        )
        # w2[e] [D, I] -> T [I, D] -> [22, 128, 1024] -> pairs [11, 128, 2048]
        a2 = w2[e].T.reshape(NI // 2, 2, 128, D)
        w2s_all.append(
            np.ascontiguousarray(a2.transpose(0, 2, 1, 3)).reshape(
                NI // 2, 128, 2 * D
            ).astype(npdt)
        )
    return w13_all, w2s_all


def kernel(x, w1, w2, w3, expert_indices, _trace=False):
    x = np.asarray(x, dtype=np.float32)
    w1 = np.asarray(w1, dtype=np.float32)
    w2 = np.asarray(w2, dtype=np.float32)
    w3 = np.asarray(w3, dtype=np.float32)
    idx = np.asarray(expert_indices).astype(np.int64)
    T, A = idx.shape
    npdt = _np_dt(DT_NAME)

    flat = idx.ravel()  # position p = t*A + a -> expert id
    order = np.argsort(flat, kind="stable")
    counts = np.bincount(flat, minlength=E)
    offs = np.zeros(E + 1, dtype=np.int64)
    np.cumsum(counts, out=offs[1:])

    w13_all, w2s_all = _prep_weights(w1, w3, w2, DT_NAME)

    out = np.empty((T * A, D), dtype=np.float32)
    remaining = counts.copy()
    done = np.zeros(E, dtype=np.int64)
    last_res = None
    while remaining.max() > 0:
        cap = min(512, max(32, int(-(-remaining.max() // 16)) * 16))
        nc = _get_program(cap, DT_NAME)
        in_maps = []
        core_pos = []  # per-core flat positions handled this round
        for e in range(E):
            n = int(min(remaining[e], cap))
            pos = order[offs[e] + done[e] : offs[e] + done[e] + n]
            core_pos.append(pos)
            xg = np.zeros((cap, D), dtype=np.float32)
            xg[:n] = x[pos // A]
            # [cap, D] -> T [D, cap] -> [k, 128, cap] -> [128, k, cap]
            xt_host = np.ascontiguousarray(
                xg.T.reshape(ND, 128, cap).transpose(1, 0, 2)
            ).reshape(128, ND * cap).astype(npdt)
            in_maps.append({"xt": xt_host, "w13": w13_all[e], "w2s": w2s_all[e]})
            remaining[e] -= n
            done[e] += n
        last_res = run_bass_kernel_spmd(
            nc, in_maps, core_ids=list(range(N_CORES)), trace=_trace
        )
        for e in range(E):
            pos = core_pos[e]
            if len(pos):
                # yt [128, 8*cap] -> [p, k, j] -> y[j, k*128+p]
                ye = (
                    last_res.results[e]["yt"]
                    .reshape(128, ND, cap)
                    .transpose(2, 1, 0)
                    .reshape(cap, D)
                )
                out[pos] = ye[: len(pos)]

    result = out.reshape(T, A, D)
    if _trace:
        return result, last_res
    return result


# revision 15
# speedup vs baseline: 1.0571x; 1.0571x over previous
"""MoE ConditionalFeedForward (SwiGLU, top-2 of 8 experts) on 8 TRN2 NeuronCores.

Strategy: expert-parallel. Core e owns expert e's weights (w1/w2/w3 slices).
The host routes tokens: for each expert, gather the tokens assigned to it
(padded to CAP), each core computes
    y = (silu(x @ w1[e].T) * (x @ w3[e].T)) @ w2[e].T
densely for its gathered tokens, and the host scatters rows back into the
[T, A, D] output.

Per-core kernel layout (all host-pretransposed so every DMA is contiguous):
  xt  [128, 8*CAP]    xt[p, k*CAP+j] = x_g[j, k*128+p]     (tokens, transposed)
  w13 [22, 128, 2048] w13[it,p,k*128+c] = w1[e][it*128+c, k*128+p], w3 at +1024
  w2s [22, 128, 1024] w2s[it,p,d] = w2[e][d, it*128+p]
  yt  [128, 8*CAP]    f32, yt[p, k*CAP+j] = y_g[j, k*128+p] (output, transposed)

Phase A (per i-tile it of 22): h1T/h3T [128(i), CAP] = sum_k wT @ x tiles in
PSUM, then hT = silu(h1)*h3 into SBUF. Phase B (transposed): yT[d-tile] [128,
CAP] accumulated over the 22 i-tiles in PSUM (8 banks, one per d-tile), with
the w2 128x128 tile stationary and hT moving.
"""

import numpy as np
from contextlib import ExitStack

import concourse.bass as bass
import concourse.bacc as bacc
import concourse.mybir as mybir
import concourse.tile as tile
from concourse.bass_utils import run_bass_kernel_spmd

E, I, D = 8, 2816, 1024
N_CORES = 8
NI, ND = I // 128, D // 128  # 22, 8

# storage dtype for weights/activations on-device: "bfloat16" (half HBM
# traffic, full PE rate) or "float32" (matmuls run as float32r, 2 cyc/row)
DT_NAME = "bfloat16"

_PROG_CACHE: dict = {}


def _build_program(cap: int, dt_name: str):
    DT = mybir.dt.float32r if dt_name == "float32" else getattr(mybir.dt, dt_name)
    f32 = mybir.dt.float32
    NP = NI // 2  # w13/w2 DMAs batched as i-tile pairs for >=1MB transfers
    nc = bacc.Bacc("TRN2", target_bir_lowering=False, debug=False)
    xt = nc.dram_tensor("xt", [128, ND * cap], DT, kind="ExternalInput").ap()
    w13 = nc.dram_tensor("w13", [NP, 128, 4 * D], DT, kind="ExternalInput").ap()
    w2s = nc.dram_tensor("w2s", [NP, 128, 2 * D], DT, kind="ExternalInput").ap()
    yt = nc.dram_tensor("yt", [128, ND * cap], f32, kind="ExternalOutput").ap()
    warm_out = nc.dram_tensor("warm_out", [128, 16], f32, kind="ExternalOutput").ap()

    with tile.TileContext(nc) as tc, ExitStack() as ctx:
        warmp = ctx.enter_context(tc.tile_pool(name="warm", bufs=1))
        xp = ctx.enter_context(tc.tile_pool(name="x", bufs=1))
        w13p = ctx.enter_context(tc.tile_pool(name="w13", bufs=4))
        hp = ctx.enter_context(tc.tile_pool(name="h", bufs=NI))
        silp = ctx.enter_context(tc.tile_pool(name="sil", bufs=2))
        w2p = ctx.enter_context(tc.tile_pool(name="w2", bufs=1))
        yp = ctx.enter_context(tc.tile_pool(name="y", bufs=1))

        # The first DMA completion sem on a ring lands ~8-9us after kernel
        # start no matter the size, so put the smallest critical piece (x's
        # k=0 slice) first on Sync, the rest of x in parallel on Scalar, and
        # stream w13 (critical path) on Sync split into per-i-tile halves.
        xsb = xp.tile([128, ND * cap], DT)
        nc.sync.dma_start(xsb[:, 0:cap], xt[:, 0:cap])
        nc.scalar.dma_start(xsb[:, cap:], xt[:, cap:])
        w13ts = []
        for j in range(4):
            wt = w13p.tile([128, 4 * D], DT, tag="w13", name=f"w13_{j}")
            nc.sync.dma_start(wt[:, 0 : 2 * D], w13[j][:, 0 : 2 * D])
            nc.sync.dma_start(wt[:, 2 * D :], w13[j][:, 2 * D :])
            w13ts.append(wt)

        # PE warmup: 8 matmuls on a zeroed tile, no DMA dependency, so the
        # HAM clock-gate is released during the initial weight-DMA window and
        # the real matmuls start at 2.4GHz.
        with tc.tile_pool(name="warmps", bufs=1, space="PSUM") as warmps:
            wtile = warmp.tile([128, 640], DT)
            nc.gpsimd.memset(wtile[:], 0.0)
            wps = warmps.tile([128, 512], f32)
            n_warm = 24
            for i in range(n_warm):
                nc.tensor.matmul(
                    wps[:],
                    wtile[:, 0:128],
                    wtile[:, 128:640],
                    start=(i == 0),
                    stop=(i == n_warm - 1),
                )
            wsc = warmp.tile([128, 16], f32)
            nc.vector.tensor_copy(wsc[:], wps[:, 0:16])
            nc.gpsimd.dma_start(warm_out[:], wsc[:])

        # w2 pair tiles on the Scalar ring: pairs 0-5 paced through phase A
        # (needed when phase B starts), pairs 6-10 paced through phase B's
        # first half, keeping phase A's DMA window under the HBM ceiling
        w2ts = [
            w2p.tile([128, 2 * D], DT, tag=f"w2_{j}", name=f"w2_{j}")
            for j in range(NP)
        ]

        hts = []
        with tc.tile_pool(name="hps", bufs=2, space="PSUM") as hps:
            for j in range(NP):
                if j < 4:
                    wt = w13ts[j]
                else:
                    wt = w13p.tile([128, 4 * D], DT, tag="w13", name=f"w13_{j}")
                    nc.sync.dma_start(wt[:, 0 : 2 * D], w13[j][:, 0 : 2 * D])
                    nc.sync.dma_start(wt[:, 2 * D :], w13[j][:, 2 * D :])
                if j % 2 == 0 and j // 2 < 6:
                    nc.scalar.dma_start(w2ts[j // 2][:], w2s[j // 2])
                for half in range(2):
                    base = half * 2 * D
                    h1 = hps.tile([128, cap], f32, tag="h1", name="h1")
                    h3 = hps.tile([128, cap], f32, tag="h3", name="h3")
                    for k in range(ND):
                        nc.tensor.matmul(
                            h1[:],
                            wt[:, base + k * 128 : base + (k + 1) * 128],
                            xsb[:, k * cap : (k + 1) * cap],
                            start=(k == 0),
                            stop=(k == ND - 1),
                        )
                    for k in range(ND):
                        nc.tensor.matmul(
                            h3[:],
                            wt[:, base + D + k * 128 : base + D + (k + 1) * 128],
                            xsb[:, k * cap : (k + 1) * cap],
                            start=(k == 0),
                            stop=(k == ND - 1),
                        )
                    sil = silp.tile([128, cap], f32)
                    nc.scalar.activation(
                        sil[:], h1[:], mybir.ActivationFunctionType.Silu
                    )
                    ht = hp.tile([128, cap], DT)
                    nc.vector.tensor_mul(ht[:], sil[:], h3[:])
                    hts.append(ht)

        # Phase B: yT[d-tile][128, cap] += w2tile.T @ hT (w2 stationary),
        # k-major within each it-half so each d-tile's PSUM drain overlaps the
        # remaining matmuls; two it-halves so w2 pairs 6-10 stream during the
        # first half
        NH = NI // 2  # 11
        with tc.tile_pool(name="yps", bufs=1, space="PSUM") as yps:
            ypt = [
                yps.tile([128, cap], f32, tag=f"yps_{k}", name=f"yps_{k}")
                for k in range(ND)
            ]
            ysb = yp.tile([128, ND * cap], f32)
            for k in range(ND):
                if 6 + k <= 10:
                    nc.scalar.dma_start(w2ts[6 + k][:], w2s[6 + k])
                for it in range(NH):
                    nc.tensor.matmul(
                        ypt[k][:],
                        w2ts[it // 2][:, (it % 2) * D + k * 128 : (it % 2) * D + (k + 1) * 128],
                        hts[it][:],
                        start=(it == 0),
                        stop=False,
                    )
            for k in range(ND):
                for it in range(NH, NI):
                    nc.tensor.matmul(
                        ypt[k][:],
                        w2ts[it // 2][:, (it % 2) * D + k * 128 : (it % 2) * D + (k + 1) * 128],
                        hts[it][:],
                        start=False,
                        stop=(it == NI - 1),
                    )
                dst = ysb[:, k * cap : (k + 1) * cap]
                nc.vector.tensor_copy(dst, ypt[k][:])
                nc.sync.dma_start(yt[:, k * cap : (k + 1) * cap], dst)

    nc.compile()
    return nc


def _get_program(cap: int, dt_name: str):
    key = (cap, dt_name)
    if key not in _PROG_CACHE:
        _PROG_CACHE[key] = _build_program(cap, dt_name)
    return _PROG_CACHE[key]


def _np_dt(dt_name: str):
    if dt_name == "float32":
        return np.float32
    import ml_dtypes

    return ml_dtypes.bfloat16


def _prep_weights(w1, w3, w2, dt_name):
    """Per-expert pretransposed/tiled weight arrays (see module docstring)."""
    npdt = _np_dt(dt_name)
    w13_all, w2s_all = [], []
    for e in range(E):
        # [I, D] -> [it, c, k, p] -> [it, p, k, c] -> [it, 128, 1024]
        a1 = w1[e].reshape(NI, 128, ND, 128).transpose(0, 3, 2, 1).reshape(NI, 128, D)
        a3 = w3[e].reshape(NI, 128, ND, 128).transpose(0, 3, 2, 1).reshape(NI, 128, D)
        # pairs of i-tiles: [11, 128, 4096] = [w1|w3] for it=2j then it=2j+1
        a13 = np.concatenate([a1, a3], axis=2).reshape(NI // 2, 2, 128, 2 * D)
        w13_all.append(
            np.ascontiguousarray(a13.transpose(0, 2, 1, 3)).reshape(
                NI // 2, 128, 4 * D
            ).astype(npdt)
        )
        # w2[e] [D, I] -> T [I, D] -> [22, 128, 1024] -> pairs [11, 128, 2048]
        a2 = w2[e].T.reshape(NI // 2, 2, 128, D)
        w2s_all.append(
            np.ascontiguousarray(a2.transpose(0, 2, 1, 3)).reshape(
                NI // 2, 128, 2 * D
            ).astype(npdt)
        )
    return w13_all, w2s_all


def kernel(x, w1, w2, w3, expert_indices, _trace=False):
    x = np.asarray(x, dtype=np.float32)
    w1 = np.asarray(w1, dtype=np.float32)
    w2 = np.asarray(w2, dtype=np.float32)
    w3 = np.asarray(w3, dtype=np.float32)
    idx = np.asarray(expert_indices).astype(np.int64)
    T, A = idx.shape
    npdt = _np_dt(DT_NAME)

    flat = idx.ravel()  # position p = t*A + a -> expert id
    order = np.argsort(flat, kind="stable")
    counts = np.bincount(flat, minlength=E)
    offs = np.zeros(E + 1, dtype=np.int64)
    np.cumsum(counts, out=offs[1:])

    w13_all, w2s_all = _prep_weights(w1, w3, w2, DT_NAME)

    out = np.empty((T * A, D), dtype=np.float32)
    remaining = counts.copy()
    done = np.zeros(E, dtype=np.int64)
    last_res = None
    while remaining.max() > 0:
        cap = min(512, max(32, int(-(-remaining.max() // 16)) * 16))
        nc = _get_program(cap, DT_NAME)
        in_maps = []
        core_pos = []  # per-core flat positions handled this round
        for e in range(E):
            n = int(min(remaining[e], cap))
            pos = order[offs[e] + done[e] : offs[e] + done[e] + n]
            core_pos.append(pos)
            xg = np.zeros((cap, D), dtype=np.float32)
            xg[:n] = x[pos // A]
            # [cap, D] -> T [D, cap] -> [k, 128, cap] -> [128, k, cap]
            xt_host = np.ascontiguousarray(
                xg.T.reshape(ND, 128, cap).transpose(1, 0, 2)
            ).reshape(128, ND * cap).astype(npdt)
            in_maps.append({"xt": xt_host, "w13": w13_all[e], "w2s": w2s_all[e]})
            remaining[e] -= n
            done[e] += n
        last_res = run_bass_kernel_spmd(
            nc, in_maps, core_ids=list(range(N_CORES)), trace=_trace
        )
        for e in range(E):
            pos = core_pos[e]
            if len(pos):
                # yt [128, 8*cap] -> [p, k, j] -> y[j, k*128+p]
                ye = (
                    last_res.results[e]["yt"]
                    .reshape(128, ND, cap)
                    .transpose(2, 1, 0)
                    .reshape(cap, D)
                )
                out[pos] = ye[: len(pos)]

    result = out.reshape(T, A, D)
    if _trace:
        return result, last_res
    return result


# revision 16
# speedup vs baseline: 1.0618x; 1.0045x over previous
"""MoE ConditionalFeedForward (SwiGLU, top-2 of 8 experts) on 8 TRN2 NeuronCores.

Strategy: expert-parallel. Core e owns expert e's weights (w1/w2/w3 slices).
The host routes tokens: for each expert, gather the tokens assigned to it
(padded to CAP), each core computes
    y = (silu(x @ w1[e].T) * (x @ w3[e].T)) @ w2[e].T
densely for its gathered tokens, and the host scatters rows back into the
[T, A, D] output.

Per-core kernel layout (all host-pretransposed so every DMA is contiguous):
  xt  [128, 8*CAP]    xt[p, k*CAP+j] = x_g[j, k*128+p]     (tokens, transposed)
  w13 [22, 128, 2048] w13[it,p,k*128+c] = w1[e][it*128+c, k*128+p], w3 at +1024
  w2s [22, 128, 1024] w2s[it,p,d] = w2[e][d, it*128+p]
  yt  [128, 8*CAP]    f32, yt[p, k*CAP+j] = y_g[j, k*128+p] (output, transposed)

Phase A (per i-tile it of 22): h1T/h3T [128(i), CAP] = sum_k wT @ x tiles in
PSUM, then hT = silu(h1)*h3 into SBUF. Phase B (transposed): yT[d-tile] [128,
CAP] accumulated over the 22 i-tiles in PSUM (8 banks, one per d-tile), with
the w2 128x128 tile stationary and hT moving.
"""

import numpy as np
from contextlib import ExitStack

import concourse.bass as bass
import concourse.bacc as bacc
import concourse.mybir as mybir
import concourse.tile as tile
from concourse.bass_utils import run_bass_kernel_spmd

E, I, D = 8, 2816, 1024
N_CORES = 8
NI, ND = I // 128, D // 128  # 22, 8

# storage dtype for weights/activations on-device: "bfloat16" (half HBM
# traffic, full PE rate) or "float32" (matmuls run as float32r, 2 cyc/row)
DT_NAME = "bfloat16"

_PROG_CACHE: dict = {}


def _build_program(cap: int, dt_name: str):
    DT = mybir.dt.float32r if dt_name == "float32" else getattr(mybir.dt, dt_name)
    f32 = mybir.dt.float32
    NP = NI // 2  # w13/w2 DMAs batched as i-tile pairs for >=1MB transfers
    nc = bacc.Bacc("TRN2", target_bir_lowering=False, debug=False)
    xt = nc.dram_tensor("xt", [128, ND * cap], DT, kind="ExternalInput").ap()
    w13 = nc.dram_tensor("w13", [NP, 128, 4 * D], DT, kind="ExternalInput").ap()
    w2s = nc.dram_tensor("w2s", [NP, 128, 2 * D], DT, kind="ExternalInput").ap()
    yt = nc.dram_tensor("yt", [128, ND * cap], f32, kind="ExternalOutput").ap()
    warm_out = nc.dram_tensor("warm_out", [128, 16], f32, kind="ExternalOutput").ap()

    with tile.TileContext(nc) as tc, ExitStack() as ctx:
        warmp = ctx.enter_context(tc.tile_pool(name="warm", bufs=1))
        xp = ctx.enter_context(tc.tile_pool(name="x", bufs=1))
        w13p = ctx.enter_context(tc.tile_pool(name="w13", bufs=4))
        hp = ctx.enter_context(tc.tile_pool(name="h", bufs=NI))
        silp = ctx.enter_context(tc.tile_pool(name="sil", bufs=3))
        w2p = ctx.enter_context(tc.tile_pool(name="w2", bufs=1))
        yp = ctx.enter_context(tc.tile_pool(name="y", bufs=1))

        # The first DMA completion sem on a ring lands ~8-9us after kernel
        # start no matter the size, so put the smallest critical piece (x's
        # k=0 slice) first on Sync, the rest of x in parallel on Scalar, and
        # stream w13 (critical path) on Sync split into per-i-tile halves.
        xsb = xp.tile([128, ND * cap], DT)
        nc.sync.dma_start(xsb[:, 0:cap], xt[:, 0:cap])
        nc.scalar.dma_start(xsb[:, cap:], xt[:, cap:])
        w13ts = []
        for j in range(4):
            wt = w13p.tile([128, 4 * D], DT, tag="w13", name=f"w13_{j}")
            nc.sync.dma_start(wt[:, 0 : 2 * D], w13[j][:, 0 : 2 * D])
            nc.sync.dma_start(wt[:, 2 * D :], w13[j][:, 2 * D :])
            w13ts.append(wt)

        # PE warmup: 8 matmuls on a zeroed tile, no DMA dependency, so the
        # HAM clock-gate is released during the initial weight-DMA window and
        # the real matmuls start at 2.4GHz.
        with tc.tile_pool(name="warmps", bufs=1, space="PSUM") as warmps:
            wtile = warmp.tile([128, 640], DT)
            nc.gpsimd.memset(wtile[:], 0.0)
            wps = warmps.tile([128, 512], f32)
            n_warm = 24
            for i in range(n_warm):
                nc.tensor.matmul(
                    wps[:],
                    wtile[:, 0:128],
                    wtile[:, 128:640],
                    start=(i == 0),
                    stop=(i == n_warm - 1),
                )
            wsc = warmp.tile([128, 16], f32)
            nc.vector.tensor_copy(wsc[:], wps[:, 0:16])
            nc.gpsimd.dma_start(warm_out[:], wsc[:])

        # w2 pair tiles on the Scalar ring: pairs 0-5 paced through phase A
        # (needed when phase B starts), pairs 6-10 paced through phase B's
        # first half, keeping phase A's DMA window under the HBM ceiling
        w2ts = [
            w2p.tile([128, 2 * D], DT, tag=f"w2_{j}", name=f"w2_{j}")
            for j in range(NP)
        ]

        hts = []
        with tc.tile_pool(name="hps", bufs=3, space="PSUM") as hps:
            for j in range(NP):
                if j < 4:
                    wt = w13ts[j]
                else:
                    wt = w13p.tile([128, 4 * D], DT, tag="w13", name=f"w13_{j}")
                    nc.sync.dma_start(wt[:, 0 : 2 * D], w13[j][:, 0 : 2 * D])
                    nc.sync.dma_start(wt[:, 2 * D :], w13[j][:, 2 * D :])
                if j % 2 == 0 and j // 2 < 6:
                    nc.scalar.dma_start(w2ts[j // 2][:], w2s[j // 2])
                for half in range(2):
                    base = half * 2 * D
                    h1 = hps.tile([128, cap], f32, tag="h1", name="h1")
                    h3 = hps.tile([128, cap], f32, tag="h3", name="h3")
                    for k in range(ND):
                        nc.tensor.matmul(
                            h1[:],
                            wt[:, base + k * 128 : base + (k + 1) * 128],
                            xsb[:, k * cap : (k + 1) * cap],
                            start=(k == 0),
                            stop=(k == ND - 1),
                        )
                    for k in range(ND):
                        nc.tensor.matmul(
                            h3[:],
                            wt[:, base + D + k * 128 : base + D + (k + 1) * 128],
                            xsb[:, k * cap : (k + 1) * cap],
                            start=(k == 0),
                            stop=(k == ND - 1),
                        )
                    sil = silp.tile([128, cap], f32)
                    nc.scalar.activation(
                        sil[:], h1[:], mybir.ActivationFunctionType.Silu
                    )
                    ht = hp.tile([128, cap], DT)
                    nc.vector.tensor_mul(ht[:], sil[:], h3[:])
                    hts.append(ht)

        # Phase B: yT[d-tile][128, cap] += w2tile.T @ hT (w2 stationary),
        # k-major within each it-half so each d-tile's PSUM drain overlaps the
        # remaining matmuls; two it-halves so w2 pairs 6-10 stream during the
        # first half
        NH = NI // 2  # 11
        with tc.tile_pool(name="yps", bufs=1, space="PSUM") as yps:
            ypt = [
                yps.tile([128, cap], f32, tag=f"yps_{k}", name=f"yps_{k}")
                for k in range(ND)
            ]
            ysb = yp.tile([128, ND * cap], f32)
            for k in range(ND):
                if 6 + k <= 10:
                    nc.scalar.dma_start(w2ts[6 + k][:], w2s[6 + k])
                for it in range(NH):
                    nc.tensor.matmul(
                        ypt[k][:],
                        w2ts[it // 2][:, (it % 2) * D + k * 128 : (it % 2) * D + (k + 1) * 128],
                        hts[it][:],
                        start=(it == 0),
                        stop=False,
                    )
            for k in range(ND):
                for it in range(NH, NI):
                    nc.tensor.matmul(
                        ypt[k][:],
                        w2ts[it // 2][:, (it % 2) * D + k * 128 : (it % 2) * D + (k + 1) * 128],
                        hts[it][:],
                        start=False,
                        stop=(it == NI - 1),
                    )
                dst = ysb[:, k * cap : (k + 1) * cap]
                nc.vector.tensor_copy(dst, ypt[k][:])
                nc.sync.dma_start(yt[:, k * cap : (k + 1) * cap], dst)

    nc.compile()
    return nc


def _get_program(cap: int, dt_name: str):
    key = (cap, dt_name)
    if key not in _PROG_CACHE:
        _PROG_CACHE[key] = _build_program(cap, dt_name)
    return _PROG_CACHE[key]


def _np_dt(dt_name: str):
    if dt_name == "float32":
        return np.float32
    import ml_dtypes

    return ml_dtypes.bfloat16


def _prep_weights(w1, w3, w2, dt_name):
    """Per-expert pretransposed/tiled weight arrays (see module docstring)."""
    npdt = _np_dt(dt_name)
    w13_all, w2s_all = [], []
    for e in range(E):
        # [I, D] -> [it, c, k, p] -> [it, p, k, c] -> [it, 128, 1024]
        a1 = w1[e].reshape(NI, 128, ND, 128).transpose(0, 3, 2, 1).reshape(NI, 128, D)
        a3 = w3[e].reshape(NI, 128, ND, 128).transpose(0, 3, 2, 1).reshape(NI, 128, D)
        # pairs of i-tiles: [11, 128, 4096] = [w1|w3] for it=2j then it=2j+1
        a13 = np.concatenate([a1, a3], axis=2).reshape(NI // 2, 2, 128, 2 * D)
        w13_all.append(
            np.ascontiguousarray(a13.transpose(0, 2, 1, 3)).reshape(
                NI // 2, 128, 4 * D
            ).astype(npdt)
        )
        # w2[e] [D, I] -> T [I, D] -> [22, 128, 1024] -> pairs [11, 128, 2048]
        a2 = w2[e].T.reshape(NI // 2, 2, 128, D)
        w2s_all.append(
            np.ascontiguousarray(a2.transpose(0, 2, 1, 3)).reshape(
                NI // 2, 128, 2 * D
            ).astype(npdt)
        )
    return w13_all, w2s_all


def kernel(x, w1, w2, w3, expert_indices, _trace=False):
    x = np.asarray(x, dtype=np.float32)
    w1 = np.asarray(w1, dtype=np.float32)
    w2 = np.asarray(w2, dtype=np.float32)
    w3 = np.asarray(w3, dtype=np.float32)
    idx = np.asarray(expert_indices).astype(np.int64)
    T, A = idx.shape
    npdt = _np_dt(DT_NAME)

    flat = idx.ravel()  # position p = t*A + a -> expert id
    order = np.argsort(flat, kind="stable")
    counts = np.bincount(flat, minlength=E)
    offs = np.zeros(E + 1, dtype=np.int64)
    np.cumsum(counts, out=offs[1:])

    w13_all, w2s_all = _prep_weights(w1, w3, w2, DT_NAME)

    out = np.empty((T * A, D), dtype=np.float32)
    remaining = counts.copy()
    done = np.zeros(E, dtype=np.int64)
    last_res = None
    while remaining.max() > 0:
        cap = min(512, max(32, int(-(-remaining.max() // 16)) * 16))
        nc = _get_program(cap, DT_NAME)
        in_maps = []
        core_pos = []  # per-core flat positions handled this round
        for e in range(E):
            n = int(min(remaining[e], cap))
            pos = order[offs[e] + done[e] : offs[e] + done[e] + n]
            core_pos.append(pos)
            xg = np.zeros((cap, D), dtype=np.float32)
            xg[:n] = x[pos // A]
            # [cap, D] -> T [D, cap] -> [k, 128, cap] -> [128, k, cap]
            xt_host = np.ascontiguousarray(
                xg.T.reshape(ND, 128, cap).transpose(1, 0, 2)
            ).reshape(128, ND * cap).astype(npdt)
            in_maps.append({"xt": xt_host, "w13": w13_all[e], "w2s": w2s_all[e]})
            remaining[e] -= n
            done[e] += n
        last_res = run_bass_kernel_spmd(
            nc, in_maps, core_ids=list(range(N_CORES)), trace=_trace
        )
        for e in range(E):
            pos = core_pos[e]
            if len(pos):
                # yt [128, 8*cap] -> [p, k, j] -> y[j, k*128+p]
                ye = (
                    last_res.results[e]["yt"]
                    .reshape(128, ND, cap)
                    .transpose(2, 1, 0)
                    .reshape(cap, D)
                )
                out[pos] = ye[: len(pos)]

    result = out.reshape(T, A, D)
    if _trace:
        return result, last_res
    return result
